# revision 26
# baseline (speedup 1.0000x reference)
"""Trainium2 Bass kernel for nn_CharStemmer (bi-LSTM encoder + LSTM decoder).

Sharding: data-parallel over batch (B=128) across 8 cores, 16 sequences per
core; all weights replicated. Inside each core:
  - the input-side gate contribution xg = emb[input] @ w_ih^T + b collapses to
    a vocab-sized contraction: EW = embedding @ w_ih^T (+ bias row) is
    precomputed on the host [72, 4H], and the per-step gate PSUM accumulation
    STARTS with a matmul of the onehot column block (K=72) against EW, then
    accumulates the 8 h-chunks against w_hh.  No xg phase, no xg DRAM traffic.
  - per step the stationary matmul operand is h^T (tiny) and w_hh^T streams
    through the PE in bf16; the 4 gate quarters are packed into the four
    32-column PE tiles and stream concurrently.
  - h^T is produced by the DMA XBAR transpose engine (16x128 tiles), not the
    PE; the gate PSUM is one wide [128, 1024] 2-bank tile per step and the
    cell elementwise runs as wide [*, 1024] ops; c lives in SBUF.
  - decoder input GEMM (ug) is interleaved one unit per decoder step after a
    one-group prologue, filling PE stalls; xq rows prefetched from DRAM.
"""

import os
import sys

for _p in ("/opt/trn_rl_repo", "/root/.axon_site/_ro/trn_rl_repo"):
    if os.path.isdir(_p) and _p not in sys.path:
        sys.path.insert(0, _p)

from contextlib import ExitStack

import ml_dtypes
import numpy as np

import concourse.bass as bass
import concourse.tile as tile
from concourse import bacc, mybir
from concourse.bass_utils import run_bass_kernel_spmd

S, B, V, E, H = 128, 128, 61, 512, 1024
NCORES = 8
BL = B // NCORES          # 16 sequences per core
G4 = 4 * H                # 4096 gate columns
VP = 72                   # vocab one-hot rows: 0:61 tokens, 64 bias row
BF16 = mybir.dt.bfloat16
F32 = mybir.dt.float32
FP8 = mybir.dt.float8e4
AF = mybir.ActivationFunctionType
ALU = mybir.AluOpType
bf16_np = ml_dtypes.bfloat16

# gate quarters in xg col order: q0=i, q1=g, q2=f, q3=o
# psum row groups: i->0:16, f->32:48, o->64:80, g->96:112
QGRP = [0, 3, 1, 2]


def _build(nc, n_steps):
    TOK = n_steps * BL

    def din(name, shape, dt):
        return nc.dram_tensor(name, list(shape), dt, kind="ExternalInput").ap()

    onehot_d = din("onehot", [VP, TOK + BL], BF16)
    ew_f_d = din("ew_f", [VP, G4], BF16)
    ew_b_d = din("ew_b", [VP, G4], BF16)
    whh_f_d = din("whh_f", [128, 8, G4], BF16)
    whh_b_d = din("whh_b", [128, 8, G4], BF16)
    wih_d_d = din("wih_d", [128, 16, G4], BF16)
    whh_d_d = din("whh_d", [128, 8, G4], BF16)
    bias_d_d = din("bias_d", [128, G4], BF16)
    decb0_d = din("decb0", [BL, G4], BF16)   # decoder step-0 gates (bias only)
    wout_d = din("wout", [128, 8, V], BF16)
    outb_d = din("outb", [V, 1], F32)
    ident_d = din("ident", [BL, 2 * BL], BF16)
    pred_d = nc.dram_tensor("pred", [V, TOK], F32, kind="ExternalOutput").ap()

    with ExitStack() as ctx:
        tc = ctx.enter_context(tile.TileContext(nc))
        dram = ctx.enter_context(tc.tile_pool(name="dram", bufs=1, space="DRAM"))
        hTf_dram = dram.tile([128, 8, TOK], BF16, tag="hTf")
        hTb_dram = dram.tile([128, 8, TOK], BF16, tag="hTb")
        hTd_dram = dram.tile([128, 8, TOK], BF16, tag="hTd")
        n_ugb = TOK // 128 if TOK >= 128 else 1
        ug_blocks = [dram.tile([min(TOK, 128), G4], BF16, tag=f"ugb{i}",
                               name=f"ug_b{i}") for i in range(n_ugb)]

        persist = ctx.enter_context(tc.tile_pool(name="persist", bufs=1))
        ident_sb = persist.tile([BL, 2 * BL], BF16, tag="ident")
        nc.sync.dma_start(ident_sb[:], ident_d[:])

        n_tok_blocks = TOK // 128 if TOK >= 128 else 1
        tok_block = min(TOK, 128)
        n_tok_chunks = TOK // 512 if TOK >= 512 else 1
        tok_chunk = min(TOK, 512)

        # ---------------- recurrence machinery -------------------------------
        class LState:
            def __init__(self, name, whh_sb, reverse, pools, ew_sb=None,
                         onehot_sb=None, hT_dram=None, xq_src=None,
                         dec_first=None):
                self.name, self.whh_sb = name, whh_sb
                self.rev = reverse
                self.ew_sb, self.onehot_sb = ew_sb, onehot_sb
                self.hT_dram = hT_dram
                self.xq_src, self.dec_first = xq_src, dec_first
                self.ew, self.psp, self.ringp = pools
                self.h_dram = hT_dram
                self.cur_rings = None
                self.h_hist = {}
                self.sig = [None, None]
                self.ps_cur = None
                self.c_sb = None
                self.xq_tiles = {}

        def emit_mm(L, s, n_steps):
            """Gate matmuls for step s: onehot/xq init + 8 w_hh chunks.

            Emission order: init (both halves), then nn0 k0..k7, nn1 k0..k7
            so the nn0 gate half completes at the midpoint and its elementwise
            chain overlaps the nn1 matmul streams."""
            t = (n_steps - 1 - s) if L.rev else s
            nm = L.name
            ps = L.psp.tile([128, 1024], F32, tag=f"ps_{nm}", bufs=2,
                            name=f"ps_{nm}_{s}")
            rp = L.cur_rings
            xq = None if L.onehot_sb is not None else L.xq_tiles.pop(t)
            for nn in range(2):
                cs = slice(nn * 512, (nn + 1) * 512)
                for q in range(4):
                    g = QGRP[q]
                    wcol = slice(q * H + nn * 512, q * H + (nn + 1) * 512)
                    if L.onehot_sb is not None:
                        nc.tensor.matmul(
                            ps[32 * g:32 * g + 32, cs],
                            L.onehot_sb[:, t * BL:t * BL + 2 * BL],
                            L.ew_sb[:, wcol],
                            start=True, stop=(s == 0), skip_group_check=True,
                            tile_position=(0, 32 * g))
                    else:
                        nc.tensor.matmul(
                            ps[32 * g:32 * g + 32, cs],
                            ident_sb[:], xq[:, 2 * q + nn, :],
                            start=True, stop=(s == 0), skip_group_check=True,
                            tile_position=(0, 32 * g))
            if s > 0:
                for nn in range(2):
                    cs = slice(nn * 512, (nn + 1) * 512)
                    for k in range(8):
                        lhsT = rp[:, k, :]
                        for q in range(4):
                            g = QGRP[q]
                            wcol = slice(q * H + nn * 512,
                                         q * H + (nn + 1) * 512)
                            nc.tensor.matmul(
                                ps[32 * g:32 * g + 16, cs],
                                lhsT, L.whh_sb[:, k, wcol],
                                start=False, stop=(k == 7),
                                skip_group_check=True,
                                tile_position=(0, 32 * g))
            L.ps_cur = ps

        def emit_sig(L, s, nn):
            """ACT front half for gate-column half nn: sigma(i,f,o) + tanh(g)."""
            nm = L.name
            ps = L.ps_cur
            cs = slice(nn * 512, (nn + 1) * 512)
            if s == 0 and nn == 0:
                L.c_full = L.ew.tile([48, H], BF16, tag=f"c_{nm}", bufs=1,
                                     name=f"c_{nm}")
                L.c_sb = L.c_full[32:48]
            sio = L.ew.tile([80, 512], BF16, tag=f"sio_{nm}{nn}", bufs=3,
                            name=f"sio_{nm}_{s}_{nn}")
            nc.scalar.activation(sio[:], ps[0:80, cs], AF.Sigmoid)
            tg = L.ew.tile([BL, 512], BF16, tag=f"tg_{nm}{nn}", bufs=2,
                           name=f"tg_{nm}_{s}_{nn}")
            nc.scalar.activation(tg[:], ps[96:112, cs], AF.Tanh)
            L.sig[nn] = (sio, tg)

        def emit_cell(L, s, nn):
            """Cell update + h + XBAR transpose for half nn.

            SB-SB operand pairs must share a base partition: c/t1/a live on
            rows 32:48 (aligned with sigma_f), zz on rows 64:80 (sigma_o)."""
            nm = L.name
            cs = slice(nn * 512, (nn + 1) * 512)
            sio, tg = L.sig[nn]
            c_h = L.c_sb[:, cs]
            if s == 0:
                nc.gpsimd.tensor_mul(c_h, sio[0:16, :], tg[:])
            else:
                t1f = L.ew.tile([48, 512], BF16, tag=f"t1_{nm}{nn}", bufs=1,
                                name=f"t1_{nm}_{s}_{nn}")
                nc.vector.tensor_mul(t1f[32:48], sio[32:48, :], c_h)
                af = L.ew.tile([48, 512], BF16, tag=f"a_{nm}{nn}", bufs=1,
                               name=f"a_{nm}_{s}_{nn}")
                nc.vector.tensor_mul(af[32:48], sio[0:16, :], tg[:])
                nc.vector.tensor_add(c_h, t1f[32:48], af[32:48])
            zzf = L.ew.tile([80, 512], BF16, tag=f"zz_{nm}{nn}", bufs=1,
                            name=f"zz_{nm}_{s}_{nn}")
            nc.scalar.activation(zzf[64:80], c_h, AF.Tanh)
            if nn == 0:
                L.h_cur = L.ew.tile([BL, H], BF16, tag=f"h_{nm}", bufs=3,
                                    name=f"h_{nm}_{s}")
            nc.vector.tensor_mul(L.h_cur[:, cs], sio[64:80, :], zzf[64:80])
            if nn == 1:
                ring = L.ringp.tile([128, 8, BL], BF16, tag=f"ring_{nm}",
                                    bufs=3, name=f"ring_{nm}_{s}")
                eng = nc.scalar if nm == "rb" else nc.sync
                eng.dma_start(ring[:], L.h_cur[:], transpose=True)
                L.cur_rings = ring
                L.h_hist[s] = ring

        def emit_h_write(L, s, n_steps):
            """DRAM spill of step s's h^T rings (delayed one step so the
            sync queue never blocks on unfinished producers)."""
            if s < 0 or L.h_dram is None:
                return
            t = (n_steps - 1 - s) if L.rev else s
            ring = L.h_hist.pop(s)
            nc.sync.dma_start(
                L.h_dram[:, :, t * BL:(t + 1) * BL], ring[:])

        # ---------------- phase R1: encoder fwd + bwd interleaved ------------
        with (
            tc.tile_pool(name="enc_in", bufs=1) as encin,
            tc.tile_pool(name="enc_whh", bufs=1) as encw,
            tc.tile_pool(name="rf_ew", bufs=1) as few,
            tc.tile_pool(name="rb_ew", bufs=1) as bew,
            tc.tile_pool(name="r_ps", bufs=1, space="PSUM") as rpsp,
            tc.tile_pool(name="r_ring", bufs=1) as ringp,
        ):
            onehot_sb = encin.tile([VP, TOK + BL], BF16, tag="onehot")
            nc.sync.dma_start(onehot_sb[:], onehot_d[:])
            ew_f_sb = encin.tile([VP, G4], BF16, tag="ewf")
            nc.sync.dma_start(ew_f_sb[:], ew_f_d[:])
            ew_b_sb = encin.tile([VP, G4], BF16, tag="ewb")
            nc.sync.dma_start(ew_b_sb[:], ew_b_d[:])
            whh_f_sb = encw.tile([128, 8, G4], BF16, tag="whhf",
                                 name="whh_f_sb")
            whh_b_sb = encw.tile([128, 8, G4], BF16, tag="whhb",
                                 name="whh_b_sb")
            for k in range(8):
                nc.sync.dma_start(whh_f_sb[:, k, :], whh_f_d[:, k, :])
                nc.sync.dma_start(whh_b_sb[:, k, :], whh_b_d[:, k, :])
            Lf = LState("rf", whh_f_sb, False, (few, rpsp, ringp),
                        ew_sb=ew_f_sb, onehot_sb=onehot_sb, hT_dram=hTf_dram)
            Lb = LState("rb", whh_b_sb, True, (bew, rpsp, ringp),
                        ew_sb=ew_b_sb, onehot_sb=onehot_sb, hT_dram=hTb_dram)
            for s in range(n_steps):
                emit_mm(Lf, s, n_steps)
                emit_sig(Lf, s, 0)
                emit_sig(Lf, s, 1)
                emit_cell(Lf, s, 0)
                emit_cell(Lf, s, 1)
                emit_mm(Lb, s, n_steps)
                emit_sig(Lb, s, 0)
                emit_sig(Lb, s, 1)
                emit_cell(Lb, s, 0)
                emit_cell(Lb, s, 1)
                emit_h_write(Lf, s - 1, n_steps)
                emit_h_write(Lb, s - 1, n_steps)
            emit_h_write(Lf, n_steps - 1, n_steps)
            emit_h_write(Lb, n_steps - 1, n_steps)

        # ---------------- phase U + R2: decoder with interleaved ug GEMM -----
        # ug[tok] = encoded[tok] @ dec_w_ih^T + dec_b  (unshifted; read at t-1)
        mg_w = min(4, n_tok_blocks)
        n_mg = max(1, n_tok_blocks // mg_w)
        mg_tok = mg_w * tok_block
        with (
            tc.tile_pool(name="dec_whh", bufs=1) as decw,
            tc.tile_pool(name="pu_w", bufs=2) as puw,
            tc.tile_pool(name="hist", bufs=2) as hist,
            tc.tile_pool(name="pu_misc", bufs=1) as pumisc,
            tc.tile_pool(name="pu_ps", bufs=4, space="PSUM") as pups,
            tc.tile_pool(name="pu_ev", bufs=4) as puev,
            tc.tile_pool(name="rd_ew", bufs=1) as dew,
            tc.tile_pool(name="rd_xq", bufs=1) as dxqp,
            tc.tile_pool(name="rd_ps", bufs=1, space="PSUM") as dpsp,
            tc.tile_pool(name="rd_ring", bufs=1) as dringp,
        ):
            whh_d_sb = decw.tile([128, 8, G4], BF16, tag="whhd",
                                 name="whh_d_sb")
            for k in range(8):
                nc.sync.dma_start(whh_d_sb[:, k, :], whh_d_d[:, k, :])
            bias_sb = pumisc.tile([128, G4], BF16, tag="biasd")
            nc.sync.dma_start(bias_sb[:], bias_d_d[:])

            wts = {}

            def load_wt(gn):
                n = gn % 8
                wt = puw.tile([128, 16, 512], BF16, tag="wt",
                              name=f"wt_{gn}")
                nc.sync.dma_start(
                    wt[:], wih_d_d[:, :, n * 512:(n + 1) * 512])
                wts[gn] = wt

            enc_hTs = {}
            ev_queue = []

            def flush_ev(keep=1):
                while len(ev_queue) > keep:
                    mb, n, ev = ev_queue.pop(0)
                    nc.sync.dma_start(
                        ug_blocks[mb][:, n * 512:(n + 1) * 512], ev[:])

            def restore(mg):
                eh = hist.tile([128, 16, mg_tok], BF16, tag="hist", bufs=1,
                               name=f"enc_hT_{mg}")
                cs = slice(mg * mg_tok, (mg + 1) * mg_tok)
                nc.sync.dma_start(eh[:, 0:8, :], hTf_dram[:, :, cs])
                nc.sync.dma_start(eh[:, 8:16, :], hTb_dram[:, :, cs])
                enc_hTs[mg] = eh

            def ug_unit(mg, n, m):
                """One (n, m) unit of the ug GEMM for block group mg."""
                gn = mg * 8 + n
                if gn not in wts:
                    load_wt(gn)
                if m == 2:   # prefetch next n-chunk / next group's hist
                    if gn + 1 < n_mg * 8:
                        load_wt(gn + 1)
                    if n == 3 and mg + 1 < n_mg:
                        restore(mg + 1)
                enc_hT = enc_hTs[mg]
                ps = pups.tile([tok_block, 512], F32, tag="ps",
                               name=f"ps_u_{gn}_{m}")
                for k in range(16):
                    nc.tensor.matmul(
                        ps[:], enc_hT[:, k, m * tok_block:(m + 1) * tok_block],
                        wts[gn][:, k, :], start=(k == 0), stop=(k == 15))
                mb = mg * mg_w + m
                ev = puev.tile([tok_block, 512], BF16, tag="ev",
                               name=f"ev_{gn}_{m}")
                nc.vector.tensor_add(
                    ev[:], ps[:], bias_sb[:tok_block, n * 512:(n + 1) * 512])
                ev_queue.append((mb, n, ev))
                if m == mg_w - 1:
                    del wts[gn]
                if mg in enc_hTs and n == 7 and m == mg_w - 1:
                    del enc_hTs[mg]

            def prefetch_xq(L, t):
                """Prefetch decoder input row block ug[t-1] into SBUF."""
                if t < 0 or t >= n_steps or t in L.xq_tiles:
                    return
                xq = dxqp.tile([BL, 8, 512], BF16, tag="xq", bufs=2,
                               name=f"xq_{t}")
                if t == 0:
                    src = decb0_d.rearrange("b (n c) -> b n c", n=8)
                else:
                    blk = tok_block // BL
                    ub = ug_blocks[(t - 1) // blk]
                    r = (t - 1) % blk
                    src = ub[r * BL:(r + 1) * BL, :].rearrange(
                        "b (n c) -> b n c", n=8)
                nc.sync.dma_start(xq[:], src)
                L.xq_tiles[t] = xq

            # prologue: first block group of ug up-front; rest interleaved
            restore(0)
            for n in range(8):
                for m in range(mg_w):
                    ug_unit(0, n, m)
                    flush_ev(keep=1)
            units = [(mg, n, m) for mg in range(1, n_mg)
                     for n in range(8) for m in range(mg_w)]

            Ld = LState("rd", whh_d_sb, False, (dew, dpsp, dringp),
                        hT_dram=hTd_dram)
            flush_ev(keep=0)
            for s in range(n_steps):
                prefetch_xq(Ld, s)
                prefetch_xq(Ld, s + 1)
                emit_mm(Ld, s, n_steps)
                emit_sig(Ld, s, 0)
                emit_sig(Ld, s, 1)
                if units:
                    ug_unit(*units.pop(0))
                emit_cell(Ld, s, 0)
                emit_cell(Ld, s, 1)
                emit_h_write(Ld, s - 1, n_steps)
                flush_ev(keep=1 if units else 0)
            for u in units:
                ug_unit(*u)
                flush_ev(keep=1)
            flush_ev(keep=0)
            emit_h_write(Ld, n_steps - 1, n_steps)

        # ---------------- phase P: vocab projection --------------------------
        with (
            tc.tile_pool(name="pp", bufs=1) as pp,
            tc.tile_pool(name="pp_h", bufs=2) as pph,
            tc.tile_pool(name="pp_ps", bufs=2, space="PSUM") as ppps,
            tc.tile_pool(name="pp_ev", bufs=2) as ppev,
        ):
            wout_sb = pp.tile([128, 8, V], BF16, tag="wout")
            nc.sync.dma_start(wout_sb[:], wout_d[:])
            outb_sb = pp.tile([V, 1], F32, tag="outb")
            nc.sync.dma_start(outb_sb[:], outb_d[:])
            for n in range(n_tok_chunks):
                hT_sb = pph.tile([128, 8, tok_chunk], BF16, tag="hT",
                                 name=f"hTp_{n}")
                nc.sync.dma_start(
                    hT_sb[:], hTd_dram[:, :, n * tok_chunk:(n + 1) * tok_chunk])
                ps = ppps.tile([V, tok_chunk], F32, tag="ps")
                for k in range(8):
                    nc.tensor.matmul(
                        ps[:], wout_sb[:, k, :], hT_sb[:, k, :],
                        start=(k == 0), stop=(k == 7))
                ev = ppev.tile([V, tok_chunk], F32, tag="ev")
                nc.vector.tensor_scalar_add(ev[:], ps[:], outb_sb[:])
                nc.sync.dma_start(
                    pred_d[:, n * tok_chunk:(n + 1) * tok_chunk], ev[:])

    return nc


_CACHE = {}


def _get_nc(n_steps):
    if n_steps not in _CACHE:
        nc = bacc.Bacc("TRN2", target_bir_lowering=False, debug=False)
        _build(nc, n_steps)
        nc.compile()
        _CACHE[n_steps] = nc
    return _CACHE[n_steps]


def _gate_perm():
    r = np.arange(G4)
    return np.concatenate([r[0:H], r[2 * H:3 * H], r[H:2 * H], r[3 * H:4 * H]])


def _prep_shared(embedding, enc_w_ih_f, enc_w_hh_f, enc_b_f, enc_w_ih_b,
                 enc_w_hh_b, enc_b_b, dec_w_ih, dec_w_hh, dec_b, out_w, out_b):
    p = _gate_perm()

    def wT(w, kt):
        return np.ascontiguousarray(
            w[p].T.reshape(kt, 128, G4).transpose(1, 0, 2).astype(bf16_np))

    def ew(w_ih, b):
        e = np.zeros((VP, G4), np.float32)
        e[:V] = embedding @ w_ih[p].T
        e[64] = b[p]
        return e.astype(bf16_np)

    shared = {
        "ew_f": ew(enc_w_ih_f, enc_b_f),
        "ew_b": ew(enc_w_ih_b, enc_b_b),
        "whh_f": wT(enc_w_hh_f, 8),
        "whh_b": wT(enc_w_hh_b, 8),
        "wih_d": wT(dec_w_ih, 16),
        "whh_d": wT(dec_w_hh, 8),
        "bias_d": np.broadcast_to(dec_b[p], (128, G4)).astype(bf16_np).copy(),
        "decb0": np.broadcast_to(dec_b[p], (BL, G4)).astype(bf16_np).copy(),
        "wout": np.ascontiguousarray(
            out_w.T.reshape(8, 128, V).transpose(1, 0, 2).astype(bf16_np)),
        "outb": out_b.reshape(V, 1).astype(np.float32),
        "ident": np.concatenate([np.eye(BL), np.zeros((BL, BL))], axis=1).astype(bf16_np),
    }
    return shared


def _in_maps(inputs, n_steps):
    input_seq = np.asarray(inputs["input_seq"]).astype(np.int64)
    shared = _prep_shared(
        *[np.asarray(inputs[k], np.float32) for k in (
            "embedding", "enc_w_ih_f", "enc_w_hh_f", "enc_b_f",
            "enc_w_ih_b", "enc_w_hh_b", "enc_b_b",
            "dec_w_ih", "dec_w_hh", "dec_b", "out_w", "out_b")])
    TOK = n_steps * BL
    in_maps = []
    for c in range(NCORES):
        idx = input_seq[:n_steps, c * BL:(c + 1) * BL]  # [n_steps, BL]
        oh = np.zeros((VP, TOK + BL), np.float32)
        cols = np.arange(TOK)
        oh[idx.reshape(-1), cols] = 1.0
        oh[64, :TOK] = 1.0
        m = dict(shared)
        m["onehot"] = oh.astype(bf16_np)
        in_maps.append(m)
    return in_maps


def _assemble(res, n_steps):
    outs = []
    for c in range(NCORES):
        pr = res.results[c]["pred"]            # [V, TOK]
        outs.append(pr.reshape(V, n_steps, BL).transpose(1, 2, 0))
    return np.concatenate(outs, axis=1).astype(np.float32)  # [n_steps, B, V]


def _run(inputs, n_steps):
    in_maps = _in_maps(inputs, n_steps)
    nc = _get_nc(n_steps)
    last_err = None
    for attempt in range(3):
        try:
            res = run_bass_kernel_spmd(nc, in_maps,
                                       core_ids=list(range(NCORES)))
            return _assemble(res, n_steps)
        except Exception as e:  # transient NRT device errors: retry
            last_err = e
            import time
            time.sleep(2.0 * (attempt + 1))
    raise last_err


def _register_ntff_hook():
    """Make antenv.axon_hooks importable (the image's antenv lacks it).

    Builds the module in-memory: a get/set hook registry plus a ctypes
    NTFF-profile hook driving /opt/axon/libaxon_pjrt.so directly
    (mirrors trn_agent_boot._ntff_profile_via_ctypes)."""
    import contextlib
    import ctypes
    import types

    if "antenv.axon_hooks" in sys.modules:
        return
    so_path = "/opt/axon/libaxon_pjrt.so"
    mod = types.ModuleType("antenv.axon_hooks")
    mod._hook = None

    def set_axon_ntff_profile_hook(h):
        mod._hook = h

    def get_axon_ntff_profile_hook():
        return mod._hook

    mod.set_axon_ntff_profile_hook = set_axon_ntff_profile_hook
    mod.get_axon_ntff_profile_hook = get_axon_ntff_profile_hook
    sys.modules["antenv.axon_hooks"] = mod
    import antenv
    antenv.axon_hooks = mod

    if not os.path.exists(so_path):
        return
    lib = ctypes.CDLL(so_path)
    if not hasattr(lib, "axon_start_nrt_profile"):
        return
    lib.axon_start_nrt_profile.argtypes = [
        ctypes.POINTER(ctypes.c_int64), ctypes.c_size_t]
    lib.axon_start_nrt_profile.restype = ctypes.c_int64
    lib.axon_stop_nrt_profile.argtypes = [ctypes.c_char_p]
    lib.axon_stop_nrt_profile.restype = ctypes.c_int64

    @contextlib.contextmanager
    def _hook(output_dir, device_ids):
        import jax
        jax.devices()
        if device_ids:
            ids = (ctypes.c_int64 * len(device_ids))(*device_ids)
            rc = lib.axon_start_nrt_profile(ids, len(device_ids))
        else:
            rc = lib.axon_start_nrt_profile(None, 0)
        if rc != 0:
            raise RuntimeError(f"axon_start_nrt_profile rc={rc}")
        try:
            yield
        finally:
            n = lib.axon_stop_nrt_profile(str(output_dir).encode())
            print(f"profile: {n} file(s) written to {output_dir}",
                  file=sys.stderr)

    mod.set_axon_ntff_profile_hook(_hook)


def _run_traced(inputs, n_steps):
    _register_ntff_hook()
    in_maps = _in_maps(inputs, n_steps)
    nc = _get_nc(n_steps)
    res = run_bass_kernel_spmd(nc, in_maps, core_ids=list(range(NCORES)),
                               trace=True)
    return _assemble(res, n_steps), res


def kernel(**inputs):
    return _run(inputs, S)
